# revision 2
# baseline (speedup 1.0000x reference)
"""Trainium2 Bass kernel v2 for sliding-window (window=256) causal attention.

Model (B=1, S=4096, H=1024, nh=16, hd=64, no q-scaling):
  q,k,v = x@wq.T, ... ; scores = q@k.T banded-causal(256); out = softmax@v @ wo.T + bo

Sharding: 2 heads per core across 8 cores (TP on head dim). Each core emits a
bf16 partial out = ctx_c @ wo[:, c].T; host sums the 8 partials and adds bo.

v2 design vs v1:
 - xT passed from host (layout transform) -> no PE transposes of x, no copies.
 - dn fused into ctx matmul: stationary [V_h | ones] gives [ctx | dn-replicated]
   in one pass (768 fewer PE rows per key tile).
 - bf16 output, bias added on host.
 - PSUM plan: scores [128,384]x4 banks, ctx+dn 2-qts-per-bank x2, shared
   512-wide pool (proj/outproj/v-transpose) x2 = exactly 8 banks, all 2-deep.
 - Explicit software pipeline: per step t emit scores(t), ctx(t-1),
   finalize(t-2); projection chunk blocks interleaved so PE never idles
   waiting for ACT's exp.
"""

import numpy as np

import concourse.bass as bass
import concourse.tile as tile
from concourse import bacc, mybir
from concourse.bass_utils import run_bass_kernel_spmd

S = 4096
H = 1024
NH = 16
HD = 64
WIN = 256
N_CORES = 8
HEADS_PER_CORE = NH // N_CORES  # 2
CD = HEADS_PER_CORE * HD  # 128
NEG = -1e30

F32 = mybir.dt.float32
F32R = mybir.dt.float32r
BF16 = mybir.dt.bfloat16
F16 = mybir.dt.float16

N_ST = S // 128  # 32 s-tiles
N_KT = H // 128  # 8 contraction tiles for projections
N_SC = S // 512  # 8 s-chunks for projections
AF = mybir.ActivationFunctionType


def build_program(taps=False):
    nc = bacc.Bacc("TRN2", target_bir_lowering=False, debug=False)
    tap_aps = {}
    if taps:
        for nm, shp, dt in (
            ("qt_d", [128, S], F16),
            ("kt_d", [128, S], F16),
            ("va_d", [128, N_ST * 256], BF16),
            ("ex_d", [128, 384], BF16),
            ("cd_d", [128, 512], F32),
            ("cn_d", [128, 128], F32),
        ):
            tap_aps[nm] = nc.dram_tensor(nm, shp, dt, kind="ExternalOutput").ap()

    xT_ap = nc.dram_tensor("xT", [H, S], F16, kind="ExternalInput").ap()
    wqT_ap = nc.dram_tensor("wqT", [H, CD], F16, kind="ExternalInput").ap()
    wkT_ap = nc.dram_tensor("wkT", [H, CD], F16, kind="ExternalInput").ap()
    wvT_ap = nc.dram_tensor("wvT", [H, CD], F16, kind="ExternalInput").ap()
    woT_ap = nc.dram_tensor("woT", [CD, H], F32R, kind="ExternalInput").ap()
    md_ap = nc.dram_tensor("mdiag", [128, 128], BF16, kind="ExternalInput").ap()
    mf_ap = nc.dram_tensor("mfar", [128, 128], BF16, kind="ExternalInput").ap()
    out_ap = nc.dram_tensor("out", [S, H], BF16, kind="ExternalOutput").ap()

    with tile.TileContext(nc) as tc:
        with (
            tc.tile_pool(name="consts", bufs=1) as consts,
            tc.tile_pool(name="big", bufs=1) as big,
            tc.tile_pool(name="xtc", bufs=4) as xtc,
            tc.tile_pool(name="pspj", bufs=1, space="PSUM") as pspj,
            tc.tile_pool(name="ps512", bufs=2, space="PSUM") as ps512,
            tc.tile_pool(name="pssc", bufs=3, space="PSUM") as pssc,
            tc.tile_pool(name="pscd", bufs=2, space="PSUM") as pscd,
            tc.tile_pool(name="expp", bufs=4) as expp,
            tc.tile_pool(name="recp", bufs=2) as recp,
            tc.tile_pool(name="ctxp", bufs=3) as ctxp,
            tc.tile_pool(name="outp", bufs=4) as outp,
        ):
            # ---------------- constants ----------------
            # DMA order tuned for startup: first half of x chunk 0, wq (Q proj
            # can start), rest of chunk 0, wk/wv, small consts, chunk 1, wo.
            xT_r = xT_ap.rearrange("(kt p) s -> p kt s", p=128)
            x_chunks = []

            def dma_x_chunk(c, split=False):
                xt = xtc.tile([128, N_KT, 512], F16, tag="xt")
                sl = xT_r[:, :, c * 512 : (c + 1) * 512]
                if split:
                    nc.sync.dma_start(out=xt[:, 0:4, :], in_=sl[:, 0:4, :])
                    return xt, (lambda: nc.sync.dma_start(out=xt[:, 4:8, :], in_=sl[:, 4:8, :]))
                nc.sync.dma_start(out=xt[:], in_=sl)
                return xt

            wq_sb = consts.tile([128, N_KT, CD], F16)
            wk_sb = consts.tile([128, N_KT, CD], F16)
            wv_sb = consts.tile([128, N_KT, CD], F16)
            xt0, xt0_rest = dma_x_chunk(0, split=True)
            x_chunks.append(xt0)
            nc.sync.dma_start(
                out=wq_sb[:], in_=wqT_ap.rearrange("(kt p) d -> p kt d", p=128)
            )
            xt0_rest()
            for w_sb, w_ap in ((wk_sb, wkT_ap), (wv_sb, wvT_ap)):
                nc.sync.dma_start(
                    out=w_sb[:], in_=w_ap.rearrange("(kt p) d -> p kt d", p=128)
                )
            md_sb = consts.tile([128, 128], BF16)
            nc.sync.dma_start(out=md_sb[:], in_=md_ap[:])
            mf_sb = consts.tile([128, 128], BF16)
            nc.sync.dma_start(out=mf_sb[:], in_=mf_ap[:])
            x_chunks.append(dma_x_chunk(1))
            wo_sb = consts.tile([128, H], F32R)
            nc.sync.dma_start(out=wo_sb[:], in_=woT_ap[:])

            # ---------------- persistent activations ----------------
            qt_sb = big.tile([128, S], F16)  # QT: [2h*64 dims, S]
            kt_sb = big.tile([128, S], F16)
            # va: per s-tile [V_h0(64) | ones(64) | V_h1(64) | ones(64)] bf16
            va = big.tile([128, N_ST, 256], BF16)
            nc.gpsimd.memset(va[:, :, 64:128], 1.0)
            nc.gpsimd.memset(va[:, :, 192:256], 1.0)

            # ---------------- emission machinery ----------------
            ctxdn = {}  # qt-pair index p -> psum tile [128, 512]

            def proj_slice(c, i):
                """Quarter of chunk c's projections: i=0/1/2 -> Q/K/V matmul
                group + psum copy; i=3 -> V-transposes + va copies. Spread
                between attention steps so ACT/DVE never fall behind PE."""
                if i == 0 and c + 2 < N_SC:
                    x_chunks.append(dma_x_chunk(c + 2))
                xt = x_chunks[c]
                if i < 2:
                    w_sb, dstT = ((wq_sb, qt_sb), (wk_sb, kt_sb))[i]
                    pps = pspj.tile([128, 512], F32, tag="pj")
                    for kt in range(N_KT):
                        nc.tensor.matmul(
                            pps[:],
                            w_sb[:, kt, :],
                            xt[:, kt, :],
                            start=(kt == 0),
                            stop=(kt == N_KT - 1),
                        )
                    dst = dstT[:, c * 512 : (c + 1) * 512]
                    nc.scalar.copy(dst, pps[:])
                else:
                    # V computed directly in [s, d] orientation: stationary =
                    # fp16 xT subtiles, moving = wvT. One psum group spans the
                    # bank (4 s-tiles x 128 dims).
                    half = i - 2  # 0: s-tiles 0-1, 1: s-tiles 2-3
                    tp = ps512.tile([128, 256], F32, tag="t512")
                    for k in range(2):
                        st4 = 2 * half + k
                        for kt in range(N_KT):
                            nc.tensor.matmul(
                                tp[:, k * 128 : (k + 1) * 128],
                                xt[:, kt, st4 * 128 : (st4 + 1) * 128],
                                wv_sb[:, kt, :],
                                start=(k == 0 and kt == 0),
                                stop=(k == 1 and kt == N_KT - 1),
                            )
                    src = tp.rearrange("p (t d) -> p t d", t=2)
                    s0 = 4 * c + 2 * half
                    nc.vector.tensor_copy(va[:, s0 : s0 + 2, 0:64], src[:, :, 0:64])
                    nc.vector.tensor_copy(
                        va[:, s0 : s0 + 2, 128:192], src[:, :, 64:128]
                    )

            def scores_block(t):
                W = min(384, (N_ST - t) * 128)
                q0 = t * 128
                sps_l = []
                for h in (0, 1):
                    sps = pssc.tile([128, 384], F32, tag="sc")
                    nc.tensor.matmul(
                        sps[:, :W],
                        kt_sb[h * 64 : (h + 1) * 64, q0 : q0 + 128],
                        qt_sb[h * 64 : (h + 1) * 64, q0 : q0 + W],
                        start=True,
                        stop=True,
                    )
                    sps_l.append(sps)
                ex_l = []
                for h in (0, 1):
                    ex = expp.tile([128, 384], BF16, tag="ex")
                    nc.scalar.activation(ex[:, :W], sps_l[h][:, :W], AF.Exp)
                    ex_l.append(ex)
                # multiplicative 0/1 band masks post-exp, off the PE: diag
                # tile on DVE (bf16 4x mode), far tile on the idle Pool engine
                # (its ctx consumer is the last j, so the latency hides).
                for h in (0, 1):
                    nc.vector.tensor_mul(
                        ex_l[h][:, 0:128], ex_l[h][:, 0:128], md_sb[:]
                    )
                if W == 384:
                    for h in (0, 1):
                        nc.gpsimd.tensor_mul(
                            ex_l[h][:, 256:384], ex_l[h][:, 256:384], mf_sb[:]
                        )
                if taps and t == 5:
                    nc.sync.dma_start(out=tap_aps["ex_d"][:], in_=ex_l[0][:])
                return ex_l

            def ctxdn_tile(qt):
                p = qt // 2
                if p not in ctxdn:
                    ctxdn[p] = pscd.tile([128, 512], F32, tag="cd", name=f"cd{p}")
                return ctxdn[p]

            def ctx_block(t, ex_l):
                W = min(384, (N_ST - t) * 128)
                for h in (0, 1):
                    st_h = va[:, t, h * 128 : (h + 1) * 128]
                    for j in range(W // 128):
                        qt = t + j
                        T = ctxdn_tile(qt)
                        b = (qt % 2) * 256
                        # one psum accumulation group per bank-tile (2 qts x
                        # 2 heads): start zeroes the whole bank at the first
                        # touch (even qt, h0, first kt); stop closes it at the
                        # last (odd qt, h1, kt==qt).
                        nc.tensor.matmul(
                            T[:, b : b + 128] if h == 0 else T[:, b + 128 : b + 256],
                            st_h,
                            ex_l[h][:, j * 128 : (j + 1) * 128],
                            start=(h == 0 and qt % 2 == 0 and t == max(qt - 2, 0)),
                            stop=(h == 1 and qt % 2 == 1 and t == qt),
                        )

            def finalize(qt):
                T = ctxdn.pop(qt // 2) if qt % 2 == 1 else ctxdn[qt // 2]
                b = (qt % 2) * 256
                if taps and qt == 5:
                    cdst = outp.tile([128, 512], F32, tag="cds", bufs=1)
                    nc.vector.tensor_copy(cdst[:], T[:])
                    nc.sync.dma_start(out=tap_aps["cd_d"][:], in_=cdst[:])
                rec = recp.tile([128, 256], F32, tag="rec")
                # dn_h0/h1 both live at partitions 64:128 of their regions:
                # one 256-wide reciprocal covers both heads
                nc.vector.reciprocal(rec[0:64, :], T[64:128, b : b + 256])
                ctxr = ctxp.tile([128, 128], F32R, tag="cn")
                nc.vector.tensor_mul(
                    ctxr[0:64, :], T[0:64, b : b + 128], rec[0:64, 0:128]
                )
                nc.vector.tensor_mul(
                    ctxr[64:128, :], T[0:64, b + 128 : b + 256], rec[0:64, 128:256]
                )
                if taps and qt == 5:
                    nc.sync.dma_start(
                        out=tap_aps["cn_d"][:], in_=ctxr.bitcast(F32)
                    )
                osb = outp.tile([128, H], BF16, tag="ob")
                for half in range(2):
                    ops = ps512.tile([128, 512], F32, tag="t512")
                    nc.tensor.matmul(
                        ops[:],
                        ctxr[:],
                        wo_sb[:, half * 512 : (half + 1) * 512],
                        start=True,
                        stop=True,
                    )
                    dst = osb[:, half * 512 : (half + 1) * 512]
                    # tail (no exps left): ACT is idle, keep DVE free to drain
                    if qt >= 28 or (qt + half) % 2 != 0:
                        nc.scalar.copy(dst, ops[:])
                    else:
                        nc.vector.tensor_copy(dst, ops[:])
                nc.sync.dma_start(out=out_ap[qt * 128 : (qt + 1) * 128, :], in_=osb[:])

            # ---------------- pipelined emission ----------------
            ex_hist = {}

            def step(t):
                if t < N_ST:
                    ex_hist[t] = scores_block(t)
                if 1 <= t <= N_ST:
                    ctx_block(t - 1, ex_hist.pop(t - 1))
                if t >= 2:
                    finalize(t - 2)

            # chunk 0 projected upfront; chunk c>=1's four slices are emitted
            # after steps 4c-6..4c-3 (clamped to >=0), i.e. done before its
            # first consumer at step 4c-2.
            # deadlines: Q(c) before step 4c-2, K(c) before 4c, V/VT(c)
            # before 4c+1; emit each ~2 steps ahead of its deadline.
            due = {}
            for c in range(1, N_SC):
                if c < N_SC - 1:
                    sched = (4 * c - 4, 4 * c - 3, 4 * c - 2, 4 * c - 1)
                else:
                    # last chunk: stretch into the pipeline tail (deadlines:
                    # Q<26, K<28, V/va<29)
                    sched = (4 * c - 4, 4 * c - 2, 4 * c - 1, 4 * c - 1)
                for i, s in enumerate(sched):
                    due.setdefault(max(s, 0), []).append((c, i))
            for i in range(4):
                proj_slice(0, i)
            for t in range(N_ST + 2):
                step(t)
                for c, i in due.get(t, ()):
                    proj_slice(c, i)

            if taps:
                nc.sync.dma_start(out=tap_aps["qt_d"][:], in_=qt_sb[:])
                nc.sync.dma_start(out=tap_aps["kt_d"][:], in_=kt_sb[:])
                nc.sync.dma_start(
                    out=tap_aps["va_d"][:],
                    in_=va.rearrange("p t d -> p (t d)"),
                )

    nc.compile()
    return nc


def build_in_maps(x, wq, wk, wv, wo, bo):
    xT = np.ascontiguousarray(x.reshape(S, H).T, dtype=np.float16)

    b = np.arange(128)[:, None]  # k (partition)
    a = np.arange(128)[None, :]  # q (free)
    mask_d = np.where(b <= a, 1.0, 0.0)  # diag tile: keep k <= q
    mask_f = np.where(b > a, 1.0, 0.0)  # far tile: keep k > q-256
    to_bf16 = lambda m: m.astype(np.float32).astype(bf16_np)

    in_maps = []
    for c in range(N_CORES):
        r0, r1 = c * CD, (c + 1) * CD
        in_maps.append(
            {
                "xT": xT,
                "wqT": np.ascontiguousarray(wq[r0:r1, :].T, dtype=np.float16),
                "wkT": np.ascontiguousarray(wk[r0:r1, :].T, dtype=np.float16),
                "wvT": np.ascontiguousarray(wv[r0:r1, :].T, dtype=np.float16),
                "woT": np.ascontiguousarray(wo[:, r0:r1].T, dtype=np.float32),
                "mdiag": to_bf16(mask_d),
                "mfar": to_bf16(mask_f),
            }
        )
    return in_maps


try:
    import ml_dtypes

    bf16_np = ml_dtypes.bfloat16
except ImportError:  # pragma: no cover
    import jax.numpy as jnp

    bf16_np = jnp.bfloat16

_NC_CACHE = None


def kernel(x, wq, wk, wv, wo, bo):
    global _NC_CACHE
    if _NC_CACHE is None:
        _NC_CACHE = build_program()
    nc = _NC_CACHE
    in_maps = build_in_maps(x, wq, wk, wv, wo, bo)
    res = run_bass_kernel_spmd(nc, in_maps, list(range(N_CORES)))
    out = res.results[0]["out"].astype(np.float64)
    for c in range(1, N_CORES):
        out += res.results[c]["out"].astype(np.float64)
    out += bo.astype(np.float64)[None, :]
    return out.reshape(1, S, H).astype(np.float32)


# revision 4
# speedup vs baseline: 1.1183x; 1.1183x over previous
"""Trainium2 Bass kernel for sliding-window (window=256) causal attention.

Model (B=1, S=4096, H=1024, nh=16, hd=64, no q-scaling):
  q,k,v = x@wq.T, ... ; scores = q@k.T banded-causal(256); out = softmax@v @ wo.T + bo

Sharding: 2 heads per core across 8 cores (TP on the head dim). Each core
emits a bf16 partial out = ctx_c @ wo[:, c-slice].T; the host sums the 8
partials and adds the bias (the all-reduce step).

Design notes:
 - x is passed pre-transposed (xT, fp16) from the host: a pure layout/dtype
   transform that removes all PE transposes of x and halves its DMA.
 - q/k in fp16 (10-bit mantissa keeps exp(q.k) accurate); exp outputs and v
   in bf16 (exp can reach e^45, beyond fp16 range); f32 psum accumulation.
 - V computed directly in [s, d] orientation (stationary = fp16 xT subtiles)
   so no V transposes are needed; va rows [V_h | ones] make one matmul emit
   both context and softmax denominator ([ctx 64p | dn-replicated 64p]).
 - Band masks applied multiplicatively post-exp, off the PE: diag tiles on
   DVE (bf16 4x mode), far tiles on the otherwise idle Pool engine.
 - PSUM plan (8 banks, one accumulation group per bank): scores 3, proj 1,
   outproj/V shared 2, ctx+dn 2 (two q-tiles packed per bank via a single
   group whose start zeroes the whole bank).
 - Explicit software pipeline over key-tiles t: scores(t) | ctx(t-1) |
   finalize(t-2), with each chunk's Q/K/V projection slices spread between
   steps (deadline-scheduled) so ACT/DVE never fall behind the PE.
"""

import numpy as np

import concourse.bass as bass
import concourse.tile as tile
from concourse import bacc, mybir
from concourse.bass_utils import run_bass_kernel_spmd

S = 4096
H = 1024
NH = 16
HD = 64
WIN = 256
N_CORES = 8
HEADS_PER_CORE = NH // N_CORES  # 2
CD = HEADS_PER_CORE * HD  # 128
NEG = -1e30

F32 = mybir.dt.float32
F32R = mybir.dt.float32r
BF16 = mybir.dt.bfloat16
F16 = mybir.dt.float16

N_ST = S // 128  # 32 s-tiles
N_KT = H // 128  # 8 contraction tiles for projections
N_SC = S // 512  # 8 s-chunks for projections
AF = mybir.ActivationFunctionType


def build_program(taps=False):
    nc = bacc.Bacc("TRN2", target_bir_lowering=False, debug=False)
    tap_aps = {}
    if taps:
        for nm, shp, dt in (
            ("qt_d", [128, S], F16),
            ("kt_d", [128, S], F16),
            ("va_d", [128, N_ST * 256], BF16),
            ("ex_d", [128, 384], BF16),
            ("cd_d", [128, 512], F32),
            ("cn_d", [128, 128], F32),
        ):
            tap_aps[nm] = nc.dram_tensor(nm, shp, dt, kind="ExternalOutput").ap()

    xT_ap = nc.dram_tensor("xT", [H, S], F16, kind="ExternalInput").ap()
    wqT_ap = nc.dram_tensor("wqT", [H, CD], F16, kind="ExternalInput").ap()
    wkT_ap = nc.dram_tensor("wkT", [H, CD], F16, kind="ExternalInput").ap()
    wvT_ap = nc.dram_tensor("wvT", [H, CD], F16, kind="ExternalInput").ap()
    woT_ap = nc.dram_tensor("woT", [CD, H], F32R, kind="ExternalInput").ap()
    md_ap = nc.dram_tensor("mdiag", [128, 128], BF16, kind="ExternalInput").ap()
    mf_ap = nc.dram_tensor("mfar", [128, 128], BF16, kind="ExternalInput").ap()
    out_ap = nc.dram_tensor("out", [S, H], BF16, kind="ExternalOutput").ap()

    with tile.TileContext(nc) as tc:
        with (
            tc.tile_pool(name="consts", bufs=1) as consts,
            tc.tile_pool(name="big", bufs=1) as big,
            tc.tile_pool(name="xtc", bufs=4) as xtc,
            tc.tile_pool(name="pspj", bufs=1, space="PSUM") as pspj,
            tc.tile_pool(name="ps512", bufs=2, space="PSUM") as ps512,
            tc.tile_pool(name="pssc", bufs=3, space="PSUM") as pssc,
            tc.tile_pool(name="pscd", bufs=2, space="PSUM") as pscd,
            tc.tile_pool(name="expp", bufs=6) as expp,
            tc.tile_pool(name="recp", bufs=3) as recp,
            tc.tile_pool(name="ctxp", bufs=4) as ctxp,
            tc.tile_pool(name="outp", bufs=6) as outp,
        ):
            # ---------------- constants ----------------
            # DMA order tuned for startup: first half of x chunk 0, wq (Q proj
            # can start), rest of chunk 0, wk/wv, small consts, chunk 1, wo.
            xT_r = xT_ap.rearrange("(kt p) s -> p kt s", p=128)
            x_chunks = []

            def dma_x_chunk(c, split=False):
                xt = xtc.tile([128, N_KT, 512], F16, tag="xt")
                sl = xT_r[:, :, c * 512 : (c + 1) * 512]
                if split:
                    nc.sync.dma_start(out=xt[:, 0:4, :], in_=sl[:, 0:4, :])
                    return xt, (lambda: nc.sync.dma_start(out=xt[:, 4:8, :], in_=sl[:, 4:8, :]))
                nc.sync.dma_start(out=xt[:], in_=sl)
                return xt

            wq_sb = consts.tile([128, N_KT, CD], F16)
            wk_sb = consts.tile([128, N_KT, CD], F16)
            wv_sb = consts.tile([128, N_KT, CD], F16)
            xt0, xt0_rest = dma_x_chunk(0, split=True)
            x_chunks.append(xt0)
            nc.sync.dma_start(
                out=wq_sb[:], in_=wqT_ap.rearrange("(kt p) d -> p kt d", p=128)
            )
            xt0_rest()
            for w_sb, w_ap in ((wk_sb, wkT_ap), (wv_sb, wvT_ap)):
                nc.sync.dma_start(
                    out=w_sb[:], in_=w_ap.rearrange("(kt p) d -> p kt d", p=128)
                )
            md_sb = consts.tile([128, 128], BF16)
            nc.sync.dma_start(out=md_sb[:], in_=md_ap[:])
            mf_sb = consts.tile([128, 128], BF16)
            nc.sync.dma_start(out=mf_sb[:], in_=mf_ap[:])
            x_chunks.append(dma_x_chunk(1))
            wo_sb = consts.tile([128, H], F32R)
            nc.sync.dma_start(out=wo_sb[:], in_=woT_ap[:])

            # ---------------- persistent activations ----------------
            qt_sb = big.tile([128, S], F16)  # QT: [2h*64 dims, S]
            kt_sb = big.tile([128, S], F16)
            # va: per s-tile [V_h0(64) | ones(64) | V_h1(64) | ones(64)] bf16
            va = big.tile([128, N_ST, 256], BF16)
            nc.gpsimd.memset(va[:, :, 64:128], 1.0)
            nc.gpsimd.memset(va[:, :, 192:256], 1.0)

            # ---------------- emission machinery ----------------
            ctxdn = {}  # qt-pair index p -> psum tile [128, 512]

            def proj_slice(c, i):
                """Quarter of chunk c's projections: i=0/1/2 -> Q/K/V matmul
                group + psum copy; i=3 -> V-transposes + va copies. Spread
                between attention steps so ACT/DVE never fall behind PE."""
                if i == 0 and c + 2 < N_SC:
                    x_chunks.append(dma_x_chunk(c + 2))
                xt = x_chunks[c]
                if i < 2:
                    w_sb, dstT = ((wq_sb, qt_sb), (wk_sb, kt_sb))[i]
                    pps = pspj.tile([128, 512], F32, tag="pj")
                    for kt in range(N_KT):
                        nc.tensor.matmul(
                            pps[:],
                            w_sb[:, kt, :],
                            xt[:, kt, :],
                            start=(kt == 0),
                            stop=(kt == N_KT - 1),
                        )
                    dst = dstT[:, c * 512 : (c + 1) * 512]
                    nc.scalar.copy(dst, pps[:])
                else:
                    # V computed directly in [s, d] orientation: stationary =
                    # fp16 xT subtiles, moving = wvT. One psum group spans the
                    # bank (4 s-tiles x 128 dims).
                    half = i - 2  # 0: s-tiles 0-1, 1: s-tiles 2-3
                    tp = ps512.tile([128, 256], F32, tag="t512")
                    for k in range(2):
                        st4 = 2 * half + k
                        for kt in range(N_KT):
                            nc.tensor.matmul(
                                tp[:, k * 128 : (k + 1) * 128],
                                xt[:, kt, st4 * 128 : (st4 + 1) * 128],
                                wv_sb[:, kt, :],
                                start=(k == 0 and kt == 0),
                                stop=(k == 1 and kt == N_KT - 1),
                            )
                    src = tp.rearrange("p (t d) -> p t d", t=2)
                    s0 = 4 * c + 2 * half
                    nc.vector.tensor_copy(va[:, s0 : s0 + 2, 0:64], src[:, :, 0:64])
                    nc.vector.tensor_copy(
                        va[:, s0 : s0 + 2, 128:192], src[:, :, 64:128]
                    )

            def scores_block(t):
                W = min(384, (N_ST - t) * 128)
                q0 = t * 128
                sps_l = []
                for h in (0, 1):
                    sps = pssc.tile([128, 384], F32, tag="sc")
                    nc.tensor.matmul(
                        sps[:, :W],
                        kt_sb[h * 64 : (h + 1) * 64, q0 : q0 + 128],
                        qt_sb[h * 64 : (h + 1) * 64, q0 : q0 + W],
                        start=True,
                        stop=True,
                    )
                    sps_l.append(sps)
                ex_l = []
                for h in (0, 1):
                    ex = expp.tile([128, 384], BF16, tag="ex")
                    nc.scalar.activation(ex[:, :W], sps_l[h][:, :W], AF.Exp)
                    ex_l.append(ex)
                # multiplicative 0/1 band masks post-exp, off the PE: diag
                # tile on DVE (bf16 4x mode), far tile on the idle Pool engine
                # (its ctx consumer is the last j, so the latency hides).
                for h in (0, 1):
                    nc.vector.tensor_mul(
                        ex_l[h][:, 0:128], ex_l[h][:, 0:128], md_sb[:]
                    )
                if W == 384:
                    for h in (0, 1):
                        nc.gpsimd.tensor_mul(
                            ex_l[h][:, 256:384], ex_l[h][:, 256:384], mf_sb[:]
                        )
                if taps and t == 5:
                    nc.sync.dma_start(out=tap_aps["ex_d"][:], in_=ex_l[0][:])
                return ex_l

            def ctxdn_tile(qt):
                p = qt // 2
                if p not in ctxdn:
                    ctxdn[p] = pscd.tile([128, 512], F32, tag="cd", name=f"cd{p}")
                return ctxdn[p]

            def ctx_block(t, ex_l):
                W = min(384, (N_ST - t) * 128)
                for h in (0, 1):
                    st_h = va[:, t, h * 128 : (h + 1) * 128]
                    for j in range(W // 128):
                        qt = t + j
                        T = ctxdn_tile(qt)
                        b = (qt % 2) * 256
                        # one psum accumulation group per bank-tile (2 qts x
                        # 2 heads): start zeroes the whole bank at the first
                        # touch (even qt, h0, first kt); stop closes it at the
                        # last (odd qt, h1, kt==qt).
                        nc.tensor.matmul(
                            T[:, b : b + 128] if h == 0 else T[:, b + 128 : b + 256],
                            st_h,
                            ex_l[h][:, j * 128 : (j + 1) * 128],
                            start=(h == 0 and qt % 2 == 0 and t == max(qt - 2, 0)),
                            stop=(h == 1 and qt % 2 == 1 and t == qt),
                        )

            def finalize(qt):
                T = ctxdn.pop(qt // 2) if qt % 2 == 1 else ctxdn[qt // 2]
                b = (qt % 2) * 256
                if taps and qt == 5:
                    cdst = outp.tile([128, 512], F32, tag="cds", bufs=1)
                    nc.vector.tensor_copy(cdst[:], T[:])
                    nc.sync.dma_start(out=tap_aps["cd_d"][:], in_=cdst[:])
                rec = recp.tile([128, 256], F32, tag="rec")
                # dn_h0/h1 both live at partitions 64:128 of their regions:
                # one 256-wide reciprocal covers both heads
                nc.vector.reciprocal(rec[0:64, :], T[64:128, b : b + 256])
                ctxr = ctxp.tile([128, 128], F32R, tag="cn")
                nc.vector.tensor_mul(
                    ctxr[0:64, :], T[0:64, b : b + 128], rec[0:64, 0:128]
                )
                nc.vector.tensor_mul(
                    ctxr[64:128, :], T[0:64, b + 128 : b + 256], rec[0:64, 128:256]
                )
                if taps and qt == 5:
                    nc.sync.dma_start(
                        out=tap_aps["cn_d"][:], in_=ctxr.bitcast(F32)
                    )
                osb = outp.tile([128, H], BF16, tag="ob")
                for half in range(2):
                    ops = ps512.tile([128, 512], F32, tag="t512")
                    nc.tensor.matmul(
                        ops[:],
                        ctxr[:],
                        wo_sb[:, half * 512 : (half + 1) * 512],
                        start=True,
                        stop=True,
                    )
                    dst = osb[:, half * 512 : (half + 1) * 512]
                    # tail (no exps left): ACT is idle, keep DVE free to drain
                    if qt >= 28 or (qt + half) % 2 != 0:
                        nc.scalar.copy(dst, ops[:])
                    else:
                        nc.vector.tensor_copy(dst, ops[:])
                nc.sync.dma_start(out=out_ap[qt * 128 : (qt + 1) * 128, :], in_=osb[:])

            # ---------------- pipelined emission ----------------
            ex_hist = {}

            def step(t):
                if t < N_ST:
                    ex_hist[t] = scores_block(t)
                if 1 <= t <= N_ST:
                    ctx_block(t - 1, ex_hist.pop(t - 1))
                if t >= 2:
                    finalize(t - 2)

            # chunk 0 projected upfront; chunk c>=1's four slices are emitted
            # after steps 4c-6..4c-3 (clamped to >=0), i.e. done before its
            # first consumer at step 4c-2.
            # deadlines: Q(c) before step 4c-2, K(c) before 4c, V/VT(c)
            # before 4c+1; emit each ~2 steps ahead of its deadline.
            due = {}
            for c in range(1, N_SC):
                if c < N_SC - 1:
                    sched = (4 * c - 4, 4 * c - 3, 4 * c - 2, 4 * c - 1)
                else:
                    # last chunk: stretch into the pipeline tail (deadlines:
                    # Q<26, K<28, V/va<29)
                    sched = (4 * c - 4, 4 * c - 2, 4 * c - 1, 4 * c - 1)
                for i, s in enumerate(sched):
                    due.setdefault(max(s, 0), []).append((c, i))
            for i in range(4):
                proj_slice(0, i)
            for t in range(N_ST + 2):
                step(t)
                for c, i in due.get(t, ()):
                    proj_slice(c, i)

            if taps:
                nc.sync.dma_start(out=tap_aps["qt_d"][:], in_=qt_sb[:])
                nc.sync.dma_start(out=tap_aps["kt_d"][:], in_=kt_sb[:])
                nc.sync.dma_start(
                    out=tap_aps["va_d"][:],
                    in_=va.rearrange("p t d -> p (t d)"),
                )

    nc.compile()
    return nc


def build_in_maps(x, wq, wk, wv, wo, bo):
    x, wq, wk, wv, wo = (np.asarray(a) for a in (x, wq, wk, wv, wo))
    xT = np.ascontiguousarray(x.reshape(S, H).T, dtype=np.float16)

    b = np.arange(128)[:, None]  # k (partition)
    a = np.arange(128)[None, :]  # q (free)
    mask_d = np.where(b <= a, 1.0, 0.0)  # diag tile: keep k <= q
    mask_f = np.where(b > a, 1.0, 0.0)  # far tile: keep k > q-256
    to_bf16 = lambda m: m.astype(np.float32).astype(bf16_np)

    in_maps = []
    for c in range(N_CORES):
        r0, r1 = c * CD, (c + 1) * CD
        in_maps.append(
            {
                "xT": xT,
                "wqT": np.ascontiguousarray(wq[r0:r1, :].T, dtype=np.float16),
                "wkT": np.ascontiguousarray(wk[r0:r1, :].T, dtype=np.float16),
                "wvT": np.ascontiguousarray(wv[r0:r1, :].T, dtype=np.float16),
                "woT": np.ascontiguousarray(wo[:, r0:r1].T, dtype=np.float32),
                "mdiag": to_bf16(mask_d),
                "mfar": to_bf16(mask_f),
            }
        )
    return in_maps


try:
    import ml_dtypes

    bf16_np = ml_dtypes.bfloat16
except ImportError:  # pragma: no cover
    import jax.numpy as jnp

    bf16_np = jnp.bfloat16

_NC_CACHE = None


def kernel(x, wq, wk, wv, wo, bo):
    global _NC_CACHE
    if _NC_CACHE is None:
        _NC_CACHE = build_program()
    nc = _NC_CACHE
    in_maps = build_in_maps(x, wq, wk, wv, wo, bo)
    res = run_bass_kernel_spmd(nc, in_maps, list(range(N_CORES)))
    out = res.results[0]["out"].astype(np.float64)
    for c in range(1, N_CORES):
        out += res.results[c]["out"].astype(np.float64)
    out += np.asarray(bo).astype(np.float64)[None, :]
    return out.reshape(1, S, H).astype(np.float32)
